# revision 77
# baseline (speedup 1.0000x reference)
"""Trainium2 Bass kernel for CIN layer:
    out[b,c,d] = sum_{h,m} W[c, h*M+m] * xk[b,h,d] * x0[b,m,d] + bias[c]

Shapes (hardcoded): x0 [512,40,64] f32, xk [512,128,64] f32,
W [128,5120] f32, b [128] f32 -> out [512,128,64] f32.

Strategy: data-parallel over batch B across 8 cores (64 batches/core).
Per core, columns are the 64*64=4096 (b,d) pairs.  The 5120-long (h,m)
contraction is split into 40 chunks of 128 rows with a mixed-radix
partition layout: chunk k=(g*8+j) covers m in the 8-wide group g (5
groups) x h in the 16-wide block j (8 blocks); partition p holds
(m = 8g + p//16, h = 16j + p%16).  Then per chunk
  outer[p, col] = xkrep_j[p, col] * x0bc_g[p, col]   (elementwise bf16)
  psum[bank]   += w3[k][p,c].T @ outer[:, bank*512:] (PE, 40-chunk accum)
xkrep_j / x0bc_g are replicated host-side (pure layout, no math).

Engine balance (HW-measured): the 21M-elem/core outer-product stream
is produced ENTIRELY on the DVE (TensorTensor bf16 2x mode: 2 elem/
lane/cycle at 0.96 GHz = 245.8 Gel/s -> ~89us/iter), adjacent same-g
chunk pairs fused into one double-width op with the shared x0 slice
broadcast along the middle dim (saves the per-op ramp overhead).  The
PE only needs 68.3us for its 320 matmuls, so it trails the DVE.
IMPORTANT NEGATIVE RESULT: offloading chunks to GpSimd/Pool (62.8
Gel/s solo) is a large net LOSS in situ -- even dep-free dummy Pool
TensorTensors alongside the DVE+PE stream blow per-iter time from
~94us to ~159us (SBUF bandwidth contention).  Keep Pool idle.

Column passes: two half-width passes, A=[0,2048) into PSUM banks 0-3,
B=[2048,4096) into banks 4-7.  MM emission is chunk-major (all 4 bank
MMs per chunk back-to-back) so the PE consumes each outer tile the
moment the DVE finishes it and the tile's buffer frees quickly --
bank-major sweeps hold buffers a whole group and stall the DVE on
tag-rotation WAR (~+9us/pass, measured).  The prologue DMA stream
(14.9MB, ~45us at 332 GB/s) is issued at half-column granularity in
first-use order on the sync queue, so pass A's operands land in the
first ~7us while pass A runs ~45us.  Dep-free scratch matmuls pad the
PE stream through the boot window (an idle gap drops the PE clock from
2.4 to 1.2 GHz for the next ~3us).  Group 0 of pass A runs as two
half-width sub-groups (banks 0-1 then 2-3) fed by quarter-granularity
loads so the first DVE op starts ~2.5us earlier.  Pass eviction
(ScalarE bias-add) is per-bank; stores go on the SYNC DGE queue --
putting them on the ScalarE queue delays the eviction chain by its
667ns/DMA config time and costs ~2us/iter (measured).  The remaining
~3us/iter of eviction overhead is a fixed sync/chain latency, NOT
contention: 256-wide, 1024-wide (bank-pair PSUM tiles), bias-via-K=1-
matmul + Copy, and bf16 eviction+store (half the bytes, host upcast)
variants ALL measured equal or slower than plain 512-wide f32
Identity+bias evicts.  Output overhead overall: ~89us/iter with no
output path (exactly the DVE roofline), ~93us with it.  Single-shot
tail: the final pass's evictions alternate ScalarE / DVE
(tensor_scalar_add carries the bias) and its stores alternate the two
DGE queues -- the DVE is idle once its last outer op retires, so this
halves the post-last-matmul chain (~0.6us, measured).

DMA-descriptor shaping: W is pre-transposed to [128, 40*128] (10KB
contiguous per partition) and the output DRAM tensor is c-major
[C, BC, D] (2KB contiguous runs); descriptors under 512B pay a 2x
DMA-time penalty.  The reps>1 build (used for steady-state timing)
is the same two-pass body inside a For_i hardware loop.
"""

import numpy as np
import ml_dtypes

B, M, H, D, C = 512, 40, 128, 64, 128
N_CORES = 8
BC = B // N_CORES          # 64 batches per core
COLS = BC * D              # 4096 (b,d) columns per core
NG = 8                     # PSUM banks
GW = COLS // NG            # 512 columns per bank
MG = 8                     # m-values per chunk group
NMG = M // MG              # 5 m-groups
HB = 128 // MG             # 16 h-values per block
NHB = H // HB              # 8 h-blocks
NCHUNK = NMG * NHB         # 40 contraction chunks

_cache = {}


def _build(reps=1, n_warm=20, n_fill=5, mm_order="chunk", use_pool=False,
           pair=True, pool_noise=False, max_pair=2, direct_store=False,
           full=False, dma2q=False, skip_out=False, skip_store=False,
           ndve_buf=5, evict_split=1, store_q="sync", bias_mm=False,
           boot_split=True, psum_pair=False, out_bf16=False, tail_par=True,
           jmaj=False):
    import contextlib

    import concourse.bacc as bacc
    import concourse.mybir as mybir
    from concourse.tile import TileContext

    f32 = mybir.dt.float32
    bf16 = mybir.dt.bfloat16

    nc = bacc.Bacc("TRN2", debug=False, num_devices=N_CORES)

    xkr_d = nc.dram_tensor("xkrep_in", [NHB, 128, COLS], bf16, kind="ExternalInput")
    x0b_d = nc.dram_tensor("x0bc_in", [NMG, 128, COLS], bf16, kind="ExternalInput")
    # pre-transposed: partition-major, 10KB contiguous per partition row
    w3_d = nc.dram_tensor("w3_in", [128, NCHUNK * C], bf16, kind="ExternalInput")
    bias_d = nc.dram_tensor("bias_in", [C, 1], f32, kind="ExternalInput")
    # bias as a single-partition row for the K=1 bias matmul (direct_store)
    biasr_d = nc.dram_tensor("biasr_in", [1, C], bf16, kind="ExternalInput")
    # c-major so each output descriptor is a contiguous (b,d) run; bf16
    # halves the evict/store traffic (host upcasts to f32 after gather)
    out_d = nc.dram_tensor("out", [C, BC, D], bf16 if out_bf16 else f32,
                           kind="ExternalOutput")

    GK = 5
    NGRP = NCHUNK // GK        # 8 groups per pass
    HCOL = COLS // 2           # 2048
    bpg = BC // NG             # 8 batches per bank

    # Per 5-chunk group: one chunk to Pool, remaining four as two adjacent
    # (same-g, j/j+1) DVE pairs.  Chosen so every group pairs cleanly.
    POOL_CHUNKS = (0, 7, 10, 15, 24, 25, 34, 35)
    GROUPS = []
    for gi in range(NGRP):
        ks = list(range(gi * GK, gi * GK + GK))
        pk = [k for k in ks if k in POOL_CHUNKS]
        assert len(pk) == 1
        rest = [k for k in ks if k != pk[0]]
        pairs = [(rest[0], rest[1]), (rest[2], rest[3])]
        for a, b in pairs:
            assert b == a + 1 and a % NHB != NHB - 1 and a // NHB == b // NHB
        GROUPS.append((pk[0], pairs))

    N_DVE_BUF = ndve_buf
    N_POOL_BUF = 3

    with TileContext(nc) as tc:
        with (
            tc.tile_pool(name="const", bufs=1) as cpool,
            tc.tile_pool(name="work", bufs=1) as wpool,
            tc.tile_pool(name="outp", bufs=1) as opool,
            tc.tile_pool(name="psum", bufs=1, space="PSUM") as ppool,
        ):
            # ---- SBUF constant tiles ----
            w3_sb = cpool.tile([128, NCHUNK * C], bf16)
            bias_sb = cpool.tile([128, 1], f32)
            biasr_sb = cpool.tile([1, C], bf16, name="biasr")
            ones_sb = cpool.tile([1, GW], bf16, name="ones1")
            xkall = cpool.tile([128, NHB * COLS], bf16, name="xkall")
            xkreps = [xkall[:, i * COLS:(i + 1) * COLS] for i in range(NHB)]
            xk3 = xkall.rearrange("p (j c) -> p j c", c=COLS)
            x0bcs = [
                cpool.tile([128, COLS], bf16, name=f"x0b{i}", tag=f"x0b{i}")
                for i in range(NMG)
            ]

            # ---- prologue DMA: half-column granularity, first-use order ---
            # Single (sync-queue) stream at full DMA bandwidth.  w3 rides in
            # three slices ordered by the chunk ranges that consume them;
            # bias is only needed by the first eviction (~40us in).
            _ldn = [0]

            def ld(kind, i, c0, c1):
                t = xkreps[i] if kind == "x" else x0bcs[i]
                src = (xkr_d if kind == "x" else x0b_d).ap()[i]
                # two DGE queues so two DMA engines stream concurrently
                eng = nc.sync if (not dma2q or _ldn[0] % 2 == 0) else nc.scalar
                _ldn[0] += 1
                eng.dma_start(out=t[:, c0:c1], in_=src[:, c0:c1])

            use_order = [("0", 0), ("x", 0), ("x", 1), ("w", 0), ("x", 2),
                         ("x", 3), ("x", 4), ("x", 5), ("x", 6), ("x", 7),
                         ("0", 1), ("w", 1), ("0", 2), ("w", 2), ("0", 3),
                         ("0", 4)]
            w_slices = [(0, 10 * C), (10 * C, 25 * C), (25 * C, NCHUNK * C)]
            nc.vector.memset(ones_sb, 1.0)
            nc.sync.dma_start(out=biasr_sb, in_=biasr_d.ap())

            def ld_w(i):
                s0, s1 = w_slices[i]
                nc.sync.dma_start(out=w3_sb[:, s0:s1], in_=w3_d.ap()[:, s0:s1])

            def emit_prologue():
                QC = HCOL // 2
                if jmaj:
                    # j-major consumption: after group A (xj0 + all five x0
                    # tiles), each group needs ONE new xj half (1.54us of
                    # DMA) vs 5.45us of DVE work -- the DMA never paces the
                    # DVE past ~14us.  w3 loads after the x0 tiles; the PE
                    # trails the DVE anyway, so its later start is free.
                    if boot_split:
                        for kind, i in [("x", 0), ("0", 0), ("0", 1),
                                        ("0", 2), ("0", 3), ("0", 4)]:
                            ld(kind, i, 0, QC)
                        for kind, i in [("x", 0), ("0", 0), ("0", 1),
                                        ("0", 2), ("0", 3), ("0", 4)]:
                            ld(kind, i, QC, HCOL)
                    else:
                        for kind, i in [("x", 0), ("0", 0), ("0", 1),
                                        ("0", 2), ("0", 3), ("0", 4)]:
                            ld(kind, i, 0, HCOL)
                    # w3 k'-order slices: [0:10C)=j0-j1, [10C:25C)=j2-j4,
                    # rest -- each lands just ahead of its consuming groups
                    ld_w(0)
                    ld("x", 1, 0, HCOL)
                    ld("x", 2, 0, HCOL)
                    ld_w(1)
                    ld("x", 3, 0, HCOL)
                    ld("x", 4, 0, HCOL)
                    ld_w(2)
                    for i in range(5, NHB):
                        ld("x", i, 0, HCOL)
                    nc.sync.dma_start(out=bias_sb, in_=bias_d.ap())
                    for kind, i in [("x", 0), ("0", 0), ("0", 1), ("0", 2),
                                    ("0", 3), ("0", 4), ("x", 1), ("x", 2),
                                    ("x", 3), ("x", 4), ("x", 5), ("x", 6),
                                    ("x", 7)]:
                        ld(kind, i, HCOL, COLS)
                    return
                if boot_split:
                    # group-0 operands at quarter granularity so the first
                    # half-width DVE op can start ~2.5us earlier
                    for kind, i in [("0", 0), ("x", 1), ("x", 2)]:
                        ld(kind, i, 0, QC)
                    ld_w(0)
                    for kind, i in [("x", 3), ("x", 4), ("x", 0)]:
                        ld(kind, i, 0, QC)
                    for kind, i in [("0", 0), ("x", 1), ("x", 2), ("x", 3),
                                    ("x", 4), ("x", 0)]:
                        ld(kind, i, QC, HCOL)
                    for kind, i in [("x", 5), ("x", 6), ("x", 7), ("0", 1)]:
                        ld(kind, i, 0, HCOL)
                    ld_w(1)
                    ld("0", 2, 0, HCOL)
                    ld_w(2)
                    ld("0", 3, 0, HCOL)
                    ld("0", 4, 0, HCOL)
                    nc.sync.dma_start(out=bias_sb, in_=bias_d.ap())
                    for kind, i in use_order:
                        if kind != "w":
                            ld(kind, i, HCOL, COLS)
                    return
                for half, (c0, c1) in enumerate([(0, HCOL), (HCOL, COLS)]):
                    for kind, i in use_order:
                        if kind == "w":
                            if half == 0:
                                ld_w(i)
                        else:
                            ld(kind, i, c0, c1)
                    if half == 0:
                        nc.sync.dma_start(out=bias_sb, in_=bias_d.ap())

            if not full:
                emit_prologue()

            loop_ctx = (
                tc.For_i(
                    0, reps, 1,
                    hint_engines=(mybir.EngineType.PE,),
                    staggered_reset=True,
                )
                if reps > 1
                else contextlib.nullcontext()
            )
            with loop_ctx:
                if full:
                    emit_prologue()
                if psum_pair:
                    # bank-pair PSUM tiles: evictions/stores run as 4 double
                    # width ops (fewer PSUM-read / SBUF-write contention
                    # windows, 4KB store descriptors)
                    pstiles = [
                        ppool.tile([128, 2 * GW], f32, name=f"psp{q}",
                                   tag=f"psp{q}")
                        for q in range(NG // 2)
                    ]
                    psums = [
                        pstiles[q // 2][:, (q % 2) * GW:(q % 2 + 1) * GW]
                        for q in range(NG)
                    ]
                else:
                    psums = [
                        ppool.tile([128, GW], f32, name=f"ps{q}", tag=f"ps{q}")
                        for q in range(NG)
                    ]

                if reps == 1 or full:
                    # Warm the PE (clock-gate needs ~3.4us of sustained
                    # activity to reach 2.4 GHz) with dummy matmuls while
                    # the first operand DMAs are in flight.  Pass A uses
                    # banks 0-3, so bank 7 absorbs the garbage; pass B's
                    # first real MM into bank 7 resets it via start=True.
                    scratch = cpool.tile([128, GW], bf16)
                    nc.scalar.memzero(scratch)
                    for _ in range(n_warm):
                        nc.tensor.matmul(
                            psums[7], lhsT=scratch[:, :128], rhs=scratch,
                            start=True, stop=True,
                        )

                ndve = 0
                npool = 0
                passes = [(0, HCOL, (0, 1, 2, 3)), (HCOL, COLS, (4, 5, 6, 7))]
                # jmaj (non-default, measured ~8us SLOWER single-shot even
                # with w3 repacked in j-major use order): group_list kept
                # only for the experiment record.  NOTE: jmaj=True requires
                # _prep_host to pack w3 as k=j*NMG+g; the default host
                # layout is row-major k=g*NHB+j, so jmaj decode below uses
                # divmod(k, NMG) against a j-major w3 — do not enable
                # without repacking.
                group_list = [
                    (None, list(range(j * NMG, (j + 1) * NMG)))
                    for j in range(NHB)
                ]
                for pi, (c0, c1, banks) in enumerate(passes):
                    width = c1 - c0
                    for gi, (pool_k, pairs) in enumerate(GROUPS):
                        if jmaj:
                            jm_ks = group_list[gi][1]
                            if (boot_split and pi == 0 and gi == 0
                                    and (reps == 1 or full)):
                                # boot group A (j=0, all g): lo/hi half-width
                                # singles, consumable on quarter loads
                                QC = width // 2
                                for half in (0, 1):
                                    cc0 = c0 + half * QC
                                    hentries = []
                                    for ui, k in enumerate(jm_ks):
                                        j2, g2 = divmod(k, NMG)
                                        t = wpool.tile(
                                            [128, QC], bf16,
                                            name=f"jbq{half}_{ui}",
                                            tag=f"jbq{ui}", bufs=1,
                                        )
                                        nc.vector.tensor_mul(
                                            t, xkreps[j2][:, cc0:cc0 + QC],
                                            x0bcs[g2][:, cc0:cc0 + QC],
                                        )
                                        hentries.append((k, t, 0))
                                    for n, (k, t, off) in enumerate(hentries):
                                        for qi2 in range(2):
                                            qb = banks[half * 2 + qi2]
                                            nc.tensor.matmul(
                                                psums[qb],
                                                lhsT=w3_sb[:, k * C:
                                                           (k + 1) * C],
                                                rhs=t[:, off + qi2 * GW:
                                                      off + (qi2 + 1) * GW],
                                                start=(n == 0),
                                                stop=False,
                                            )
                                for _ in range(n_fill):
                                    nc.tensor.matmul(
                                        psums[7], lhsT=scratch[:, :128],
                                        rhs=scratch, start=True, stop=True,
                                    )
                                continue
                            entries = []
                            for k in jm_ks:
                                j2, g2 = divmod(k, NMG)
                                t = wpool.tile(
                                    [128, width], bf16,
                                    name=f"jod{pi}_{gi}_{k}",
                                    tag=f"od1_{ndve % 5}", bufs=1,
                                )
                                ndve += 1
                                nc.vector.tensor_mul(
                                    t, xkreps[j2][:, c0:c1],
                                    x0bcs[g2][:, c0:c1],
                                )
                                entries.append((k, t, 0))
                            ne = len(entries)
                            for n, (k, t, off) in enumerate(entries):
                                for qi, qb in enumerate(banks):
                                    nc.tensor.matmul(
                                        psums[qb],
                                        lhsT=w3_sb[:, k * C:(k + 1) * C],
                                        rhs=t[:, off + qi * GW:
                                              off + (qi + 1) * GW],
                                        start=(gi == 0 and n == 0),
                                        stop=(gi == NGRP - 1 and n == ne - 1),
                                    )
                            if (reps == 1 or full) and pi == 0 and gi < 6:
                                for _ in range(n_fill):
                                    nc.tensor.matmul(
                                        psums[7], lhsT=scratch[:, :128],
                                        rhs=scratch, start=True, stop=True,
                                    )
                            continue
                        if (boot_split and pi == 0 and gi == 0
                                and (reps == 1 or full)):
                            # boot group: two half-width sub-groups (banks
                            # 0-1 then 2-3) so compute starts on quarter
                            # loads; tags reused lo->hi (WAR is benign --
                            # the hi operands arrive later anyway)
                            QC = width // 2
                            for half in (0, 1):
                                cc0 = c0 + half * QC
                                hentries = []
                                for ui, (ka, kb) in enumerate(pairs):
                                    g2, j2 = divmod(ka, NHB)
                                    t = wpool.tile(
                                        [128, 2 * QC], bf16,
                                        name=f"bqp{half}_{ui}",
                                        tag=f"bqp{ui}", bufs=1,
                                    )
                                    nc.vector.tensor_mul(
                                        t.rearrange("p (u c) -> p u c", u=2),
                                        xk3[:, j2:j2 + 2, cc0:cc0 + QC],
                                        x0bcs[g2][:, cc0:cc0 + QC]
                                        .unsqueeze(1)
                                        .broadcast_to([128, 2, QC]),
                                    )
                                    hentries.append((ka, t, 0))
                                    hentries.append((kb, t, QC))
                                g2, j2 = divmod(pool_k, NHB)
                                ts = wpool.tile(
                                    [128, QC], bf16, name=f"bqs{half}",
                                    tag="bqs", bufs=1,
                                )
                                nc.vector.tensor_mul(
                                    ts, xkreps[j2][:, cc0:cc0 + QC],
                                    x0bcs[g2][:, cc0:cc0 + QC],
                                )
                                hentries.append((pool_k, ts, 0))
                                for n, (k, t, off) in enumerate(hentries):
                                    for qi2 in range(2):
                                        qb = banks[half * 2 + qi2]
                                        nc.tensor.matmul(
                                            psums[qb],
                                            lhsT=w3_sb[:, k * C:(k + 1) * C],
                                            rhs=t[:, off + qi2 * GW:
                                                  off + (qi2 + 1) * GW],
                                            start=(n == 0),
                                            stop=False,
                                        )
                            if reps == 1 or full:
                                for _ in range(n_fill):
                                    nc.tensor.matmul(
                                        psums[7], lhsT=scratch[:, :128],
                                        rhs=scratch, start=True, stop=True,
                                    )
                            continue
                        entries = []
                        # Pool chunk first so the slow engine's stream is
                        # maximally early; 3 rotating bufs let it run ahead.
                        if use_pool or pool_noise:
                            g, j = divmod(pool_k, NHB)
                            po = wpool.tile(
                                [128, width], bf16, name=f"po{pi}_{gi}",
                                tag=f"po{npool % N_POOL_BUF}", bufs=1,
                            )
                            npool += 1
                            nc.gpsimd.tensor_mul(
                                po, xkreps[j][:, c0:c1], x0bcs[g][:, c0:c1]
                            )
                        dve_ks = []
                        for ka, kb in pairs:
                            dve_ks.extend([ka, kb])
                        if not use_pool:
                            dve_ks.append(pool_k)
                            dve_ks.sort()
                        if pair:
                            # greedy runs of adjacent same-g chunks, up to
                            # max_pair wide: one DVE op per run with the x0
                            # slice broadcast along the run dim
                            units = []
                            i = 0
                            while i < len(dve_ks):
                                k = dve_ks[i]
                                run = 1
                                while (run < max_pair
                                       and i + run < len(dve_ks)
                                       and dve_ks[i + run] == k + run
                                       and (k + run) % NHB != 0):
                                    run += 1
                                units.append((k, run))
                                i += run
                        else:
                            units = [(k, 1) for k in dve_ks]
                        for k, nun in units:
                            g2, j2 = divmod(k, NHB)
                            nbuf = (N_DVE_BUF if nun == 2
                                    else (4 if nun == 1 else 3))
                            t = wpool.tile(
                                [128, nun * width], bf16,
                                name=f"od{pi}_{gi}_{k}",
                                tag=f"od{nun}_{ndve % nbuf}", bufs=1,
                            )
                            ndve += 1
                            if nun > 1:
                                nc.vector.tensor_mul(
                                    t.rearrange("p (u c) -> p u c", u=nun),
                                    xk3[:, j2:j2 + nun, c0:c1],
                                    x0bcs[g2][:, c0:c1]
                                    .unsqueeze(1)
                                    .broadcast_to([128, nun, width]),
                                )
                                for u in range(nun):
                                    entries.append((k + u, t, u * width))
                            else:
                                nc.vector.tensor_mul(
                                    t, xkreps[j2][:, c0:c1],
                                    x0bcs[g2][:, c0:c1],
                                )
                                entries.append((k, t, 0))
                        # Pool chunk consumed last: it was produced with ~1
                        # group of lag.  Chunk-major MM order (all banks per
                        # chunk) so the PE consumes each chunk the moment it
                        # is produced instead of stalling a whole bank sweep
                        # on the last chunk of the group; also reuses the
                        # stationary w3 chunk across the 4 bank MMs.
                        if use_pool:
                            entries.append((pool_k, po, 0))
                        ne = len(entries)
                        if mm_order == "chunk":
                            order = [(n, qi) for n in range(ne)
                                     for qi in range(len(banks))]
                        else:
                            order = [(n, qi) for qi in range(len(banks))
                                     for n in range(ne)]
                        for n, qi in order:
                            k, t, off = entries[n]
                            qb = banks[qi]
                            nc.tensor.matmul(
                                psums[qb],
                                lhsT=w3_sb[:, k * C:(k + 1) * C],
                                rhs=t[:, off + qi * GW:
                                      off + (qi + 1) * GW],
                                start=(gi == 0 and n == 0),
                                stop=(not (direct_store or bias_mm)
                                      and gi == NGRP - 1 and n == ne - 1),
                            )
                        if (reps == 1 or full) and pi == 0 and gi < 6:
                            # bootstrap filler: the prologue DMA stream can
                            # momentarily starve the PE here; dep-free
                            # scratch matmuls into the not-yet-active bank 7
                            # absorb the stall (an idle gap resets the clock
                            # ramp, costing ~2x on the next ~3us of MMs)
                            for _ in range(n_fill):
                                nc.tensor.matmul(
                                    psums[7], lhsT=scratch[:, :128],
                                    rhs=scratch, start=True, stop=True,
                                )
                    if skip_out:
                        pass
                    elif direct_store:
                        # bias folded into a K=1 matmul (the stop MM of each
                        # bank), then DMA the PSUM bank straight to DRAM:
                        # no ScalarE eviction, no SBUF store traffic.
                        for qi, qb in enumerate(banks):
                            nc.tensor.matmul(
                                psums[qb],
                                lhsT=biasr_sb[0:1, :],
                                rhs=ones_sb[0:1, :],
                                start=False,
                                stop=True,
                            )
                            nc.scalar.dma_start(
                                out=out_d.ap()[:, qb * bpg:(qb + 1) * bpg, :],
                                in_=psums[qb],
                            )
                    else:
                        # bias-add eviction per bank on ScalarE; stores ride
                        # the ScalarE DGE queue so they never queue behind
                        # loads.
                        store_eng = nc.scalar if store_q == "scalar" else nc.sync
                        if psum_pair and not bias_mm and evict_split == 1:
                            for qp in range(len(banks) // 2):
                                qb = banks[2 * qp]
                                out_sb = opool.tile(
                                    [128, 2 * GW], f32, name=f"osbp{pi}{qb}",
                                    tag=f"osbp{qp}",
                                )
                                nc.scalar.activation(
                                    out_sb,
                                    pstiles[qb // 2],
                                    mybir.ActivationFunctionType.Identity,
                                    bias=bias_sb[:, 0:1],
                                    scale=1.0,
                                )
                                if not skip_store:
                                    store_eng.dma_start(
                                        out=out_d.ap()[:, qb * bpg:
                                                       (qb + 2) * bpg, :],
                                        in_=out_sb,
                                    )
                            continue_evict = False
                        else:
                            continue_evict = True
                        out_dt = bf16 if out_bf16 else f32
                        # single-shot tail: the DVE is idle once its last op
                        # retires, so split the final pass's evictions
                        # between ScalarE and DVE and its stores across both
                        # DGE queues -- halves the post-last-matmul chain
                        tail_mode = (tail_par and (reps == 1 or full)
                                     and pi == len(passes) - 1)
                        for qi, qb in enumerate(banks):
                            if not continue_evict:
                                break
                            if bias_mm:
                                # fold bias into the bank's stop MM (K=1)
                                nc.tensor.matmul(
                                    psums[qb],
                                    lhsT=biasr_sb[0:1, :],
                                    rhs=ones_sb[0:1, :],
                                    start=False,
                                    stop=True,
                                )
                            out_sb = opool.tile(
                                [128, GW], out_dt, name=f"osb{pi}{qb}",
                                tag=f"osb{qi}",
                            )
                            if tail_mode and qi % 2 == 1:
                                nc.vector.tensor_scalar_add(
                                    out_sb, psums[qb], bias_sb[:, 0:1]
                                )
                                nc.scalar.dma_start(
                                    out=out_d.ap()[:, qb * bpg:
                                                   (qb + 1) * bpg, :],
                                    in_=out_sb,
                                )
                                continue
                            es = GW // evict_split
                            for v in range(evict_split):
                                if bias_mm:
                                    nc.scalar.activation(
                                        out_sb[:, v * es:(v + 1) * es],
                                        psums[qb][:, v * es:(v + 1) * es],
                                        mybir.ActivationFunctionType.Copy,
                                    )
                                else:
                                    nc.scalar.activation(
                                        out_sb[:, v * es:(v + 1) * es],
                                        psums[qb][:, v * es:(v + 1) * es],
                                        mybir.ActivationFunctionType.Identity,
                                        bias=bias_sb[:, 0:1],
                                        scale=1.0,
                                    )
                            if not skip_store:
                                store_eng.dma_start(
                                    out=out_d.ap()[:, qb * bpg:(qb + 1) * bpg, :],
                                    in_=out_sb,
                                )

    nc.compile()
    return nc


def _prep_host(x0, xk, W, b):
    """Host-side layout prep (no arithmetic): shard, transpose, replicate."""
    part = np.arange(128)
    hh = (part % HB)[None, :] + HB * np.arange(NHB)[:, None]   # [NHB, 128]
    mm = (part // HB)[None, :] + MG * np.arange(NMG)[:, None]  # [NMG, 128]

    Wr = W.reshape(C, H, M)
    w3 = np.empty((128, NCHUNK, C), ml_dtypes.bfloat16)
    for g in range(NMG):
        for j in range(NHB):
            w3[:, g * NHB + j, :] = Wr[:, hh[j], mm[g]].T.astype(
                ml_dtypes.bfloat16
            )
    w3 = np.ascontiguousarray(w3.reshape(128, NCHUNK * C))
    bias = np.ascontiguousarray(b.reshape(C, 1)).astype(np.float32)

    in_maps = []
    for k in range(N_CORES):
        x0s = x0[k * BC:(k + 1) * BC]            # [BC, M, D]
        xks = xk[k * BC:(k + 1) * BC]            # [BC, H, D]
        xk2 = (
            np.ascontiguousarray(xks.transpose(1, 0, 2))
            .reshape(H, COLS)
            .astype(ml_dtypes.bfloat16)
        )
        x02 = (
            np.ascontiguousarray(x0s.transpose(1, 0, 2))
            .reshape(M, COLS)
            .astype(ml_dtypes.bfloat16)
        )
        in_maps.append(
            {
                "xkrep_in": np.ascontiguousarray(xk2[hh]),
                "x0bc_in": np.ascontiguousarray(x02[mm]),
                "w3_in": w3,
                "bias_in": bias,
                "biasr_in": np.ascontiguousarray(
                    b.reshape(1, C)
                ).astype(ml_dtypes.bfloat16),
            }
        )
    return in_maps


def _run(in_maps, **kwargs):
    from concourse import bass_utils

    if "nc" not in _cache:
        _cache["nc"] = _build()
    return bass_utils.run_bass_kernel_spmd(
        _cache["nc"], in_maps, core_ids=list(range(N_CORES)), **kwargs
    )


def kernel(x0, xk, W, b, _bench=[None]):
    x0 = np.asarray(x0, dtype=np.float32)
    xk = np.asarray(xk, dtype=np.float32)
    W = np.asarray(W, dtype=np.float32)
    b = np.asarray(b, dtype=np.float32)
    in_maps = _prep_host(x0, xk, W, b)
    res = _run(in_maps)
    _bench[0] = res
    # per-core out is c-major [C, BC, D] (bf16); restore [BC, C, D], stack
    # cores, upcast to f32 on host
    out = np.concatenate(
        [np.transpose(np.asarray(r["out"], dtype=np.float32), (1, 0, 2))
         for r in res.results],
        axis=0,
    )
    return np.ascontiguousarray(out, dtype=np.float32)


# revision 80
# speedup vs baseline: 1.0014x; 1.0014x over previous
"""Trainium2 Bass kernel for CIN layer:
    out[b,c,d] = sum_{h,m} W[c, h*M+m] * xk[b,h,d] * x0[b,m,d] + bias[c]

Shapes (hardcoded): x0 [512,40,64] f32, xk [512,128,64] f32,
W [128,5120] f32, b [128] f32 -> out [512,128,64] f32.

Strategy: data-parallel over batch B across 8 cores (64 batches/core).
Per core, columns are the 64*64=4096 (b,d) pairs.  The 5120-long (h,m)
contraction is split into 40 chunks of 128 rows with a mixed-radix
partition layout: chunk k=(g*8+j) covers m in the 8-wide group g (5
groups) x h in the 16-wide block j (8 blocks); partition p holds
(m = 8g + p//16, h = 16j + p%16).  Then per chunk
  outer[p, col] = xkrep_j[p, col] * x0bc_g[p, col]   (elementwise bf16)
  psum[bank]   += w3[k][p,c].T @ outer[:, bank*512:] (PE, 40-chunk accum)
xkrep_j / x0bc_g are replicated host-side (pure layout, no math).

Engine balance (HW-measured): the 21M-elem/core outer-product stream
is produced ENTIRELY on the DVE (TensorTensor bf16 2x mode: 2 elem/
lane/cycle at 0.96 GHz = 245.8 Gel/s -> ~89us/iter), adjacent same-g
chunk pairs fused into one double-width op with the shared x0 slice
broadcast along the middle dim (saves the per-op ramp overhead).  The
PE only needs 68.3us for its 320 matmuls, so it trails the DVE.
IMPORTANT NEGATIVE RESULT: offloading chunks to GpSimd/Pool (62.8
Gel/s solo) is a large net LOSS in situ -- even dep-free dummy Pool
TensorTensors alongside the DVE+PE stream blow per-iter time from
~94us to ~159us (SBUF bandwidth contention).  Keep Pool idle.

Column passes: two half-width passes, A=[0,2048) into PSUM banks 0-3,
B=[2048,4096) into banks 4-7.  MM emission is chunk-major (all 4 bank
MMs per chunk back-to-back) so the PE consumes each outer tile the
moment the DVE finishes it and the tile's buffer frees quickly --
bank-major sweeps hold buffers a whole group and stall the DVE on
tag-rotation WAR (~+9us/pass, measured).  The prologue DMA stream
(14.9MB, ~45us at 332 GB/s) is issued at half-column granularity in
first-use order on the sync queue, so pass A's operands land in the
first ~7us while pass A runs ~45us.  NO warmup/filler scratch matmuls:
the cost model's PE clock-ramp penalty (idle gap -> 1.2 GHz for ~3us)
does NOT materialize on real HW -- interleaved A/B measures every
scratch MM as a net loss (~3.8us single-shot for 50 of them), since
the in-order PE queue delays real MMs behind them and their SBUF
reads contend.  Group 0 of pass A runs as two
half-width sub-groups (banks 0-1 then 2-3) fed by quarter-granularity
loads so the first DVE op starts ~2.5us earlier.  Pass eviction
(ScalarE bias-add) is per-bank; stores go on the SYNC DGE queue --
putting them on the ScalarE queue delays the eviction chain by its
667ns/DMA config time and costs ~2us/iter (measured).  The remaining
~3us/iter of eviction overhead is a fixed sync/chain latency, NOT
contention: 256-wide, 1024-wide (bank-pair PSUM tiles), bias-via-K=1-
matmul + Copy, and bf16 eviction+store (half the bytes, host upcast)
variants ALL measured equal or slower than plain 512-wide f32
Identity+bias evicts.  Output overhead overall: ~89us/iter with no
output path (exactly the DVE roofline), ~93us with it.  Single-shot
tail: the final pass's evictions alternate ScalarE / DVE
(tensor_scalar_add carries the bias) and its stores alternate the two
DGE queues -- the DVE is idle once its last outer op retires, so this
halves the post-last-matmul chain (~0.6us, measured).

DMA-descriptor shaping: W is pre-transposed to [128, 40*128] (10KB
contiguous per partition) and the output DRAM tensor is c-major
[C, BC, D] (2KB contiguous runs); descriptors under 512B pay a 2x
DMA-time penalty.  The reps>1 build (used for steady-state timing)
is the same two-pass body inside a For_i hardware loop.
"""

import numpy as np
import ml_dtypes

B, M, H, D, C = 512, 40, 128, 64, 128
N_CORES = 8
BC = B // N_CORES          # 64 batches per core
COLS = BC * D              # 4096 (b,d) columns per core
NG = 8                     # PSUM banks
GW = COLS // NG            # 512 columns per bank
MG = 8                     # m-values per chunk group
NMG = M // MG              # 5 m-groups
HB = 128 // MG             # 16 h-values per block
NHB = H // HB              # 8 h-blocks
NCHUNK = NMG * NHB         # 40 contraction chunks

_cache = {}


def _build(reps=1, n_warm=0, n_fill=0, mm_order="chunk", use_pool=False,
           pair=True, pool_noise=False, max_pair=2, direct_store=False,
           full=False, dma2q=False, skip_out=False, skip_store=False,
           ndve_buf=5, evict_split=1, store_q="sync", bias_mm=False,
           boot_split=True, psum_pair=False, out_bf16=False, tail_par=True,
           jmaj=False):
    import contextlib

    import concourse.bacc as bacc
    import concourse.mybir as mybir
    from concourse.tile import TileContext

    f32 = mybir.dt.float32
    bf16 = mybir.dt.bfloat16

    nc = bacc.Bacc("TRN2", debug=False, num_devices=N_CORES)

    xkr_d = nc.dram_tensor("xkrep_in", [NHB, 128, COLS], bf16, kind="ExternalInput")
    x0b_d = nc.dram_tensor("x0bc_in", [NMG, 128, COLS], bf16, kind="ExternalInput")
    # pre-transposed: partition-major, 10KB contiguous per partition row
    w3_d = nc.dram_tensor("w3_in", [128, NCHUNK * C], bf16, kind="ExternalInput")
    bias_d = nc.dram_tensor("bias_in", [C, 1], f32, kind="ExternalInput")
    # bias as a single-partition row for the K=1 bias matmul (direct_store)
    biasr_d = nc.dram_tensor("biasr_in", [1, C], bf16, kind="ExternalInput")
    # c-major so each output descriptor is a contiguous (b,d) run; bf16
    # halves the evict/store traffic (host upcasts to f32 after gather)
    out_d = nc.dram_tensor("out", [C, BC, D], bf16 if out_bf16 else f32,
                           kind="ExternalOutput")

    GK = 5
    NGRP = NCHUNK // GK        # 8 groups per pass
    HCOL = COLS // 2           # 2048
    bpg = BC // NG             # 8 batches per bank

    # Per 5-chunk group: one chunk to Pool, remaining four as two adjacent
    # (same-g, j/j+1) DVE pairs.  Chosen so every group pairs cleanly.
    POOL_CHUNKS = (0, 7, 10, 15, 24, 25, 34, 35)
    GROUPS = []
    for gi in range(NGRP):
        ks = list(range(gi * GK, gi * GK + GK))
        pk = [k for k in ks if k in POOL_CHUNKS]
        assert len(pk) == 1
        rest = [k for k in ks if k != pk[0]]
        pairs = [(rest[0], rest[1]), (rest[2], rest[3])]
        for a, b in pairs:
            assert b == a + 1 and a % NHB != NHB - 1 and a // NHB == b // NHB
        GROUPS.append((pk[0], pairs))

    N_DVE_BUF = ndve_buf
    N_POOL_BUF = 3

    with TileContext(nc) as tc:
        with (
            tc.tile_pool(name="const", bufs=1) as cpool,
            tc.tile_pool(name="work", bufs=1) as wpool,
            tc.tile_pool(name="outp", bufs=1) as opool,
            tc.tile_pool(name="psum", bufs=1, space="PSUM") as ppool,
        ):
            # ---- SBUF constant tiles ----
            w3_sb = cpool.tile([128, NCHUNK * C], bf16)
            bias_sb = cpool.tile([128, 1], f32)
            biasr_sb = cpool.tile([1, C], bf16, name="biasr")
            ones_sb = cpool.tile([1, GW], bf16, name="ones1")
            xkall = cpool.tile([128, NHB * COLS], bf16, name="xkall")
            xkreps = [xkall[:, i * COLS:(i + 1) * COLS] for i in range(NHB)]
            xk3 = xkall.rearrange("p (j c) -> p j c", c=COLS)
            x0bcs = [
                cpool.tile([128, COLS], bf16, name=f"x0b{i}", tag=f"x0b{i}")
                for i in range(NMG)
            ]

            # ---- prologue DMA: half-column granularity, first-use order ---
            # Single (sync-queue) stream at full DMA bandwidth.  w3 rides in
            # three slices ordered by the chunk ranges that consume them;
            # bias is only needed by the first eviction (~40us in).
            _ldn = [0]

            def ld(kind, i, c0, c1):
                t = xkreps[i] if kind == "x" else x0bcs[i]
                src = (xkr_d if kind == "x" else x0b_d).ap()[i]
                # two DGE queues so two DMA engines stream concurrently
                eng = nc.sync if (not dma2q or _ldn[0] % 2 == 0) else nc.scalar
                _ldn[0] += 1
                eng.dma_start(out=t[:, c0:c1], in_=src[:, c0:c1])

            use_order = [("0", 0), ("x", 0), ("x", 1), ("w", 0), ("x", 2),
                         ("x", 3), ("x", 4), ("x", 5), ("x", 6), ("x", 7),
                         ("0", 1), ("w", 1), ("0", 2), ("w", 2), ("0", 3),
                         ("0", 4)]
            w_slices = [(0, 10 * C), (10 * C, 25 * C), (25 * C, NCHUNK * C)]
            nc.vector.memset(ones_sb, 1.0)
            nc.sync.dma_start(out=biasr_sb, in_=biasr_d.ap())

            def ld_w(i):
                s0, s1 = w_slices[i]
                nc.sync.dma_start(out=w3_sb[:, s0:s1], in_=w3_d.ap()[:, s0:s1])

            def emit_prologue():
                QC = HCOL // 2
                if jmaj:
                    # j-major consumption: after group A (xj0 + all five x0
                    # tiles), each group needs ONE new xj half (1.54us of
                    # DMA) vs 5.45us of DVE work -- the DMA never paces the
                    # DVE past ~14us.  w3 loads after the x0 tiles; the PE
                    # trails the DVE anyway, so its later start is free.
                    if boot_split:
                        for kind, i in [("x", 0), ("0", 0), ("0", 1),
                                        ("0", 2), ("0", 3), ("0", 4)]:
                            ld(kind, i, 0, QC)
                        for kind, i in [("x", 0), ("0", 0), ("0", 1),
                                        ("0", 2), ("0", 3), ("0", 4)]:
                            ld(kind, i, QC, HCOL)
                    else:
                        for kind, i in [("x", 0), ("0", 0), ("0", 1),
                                        ("0", 2), ("0", 3), ("0", 4)]:
                            ld(kind, i, 0, HCOL)
                    # w3 k'-order slices: [0:10C)=j0-j1, [10C:25C)=j2-j4,
                    # rest -- each lands just ahead of its consuming groups
                    ld_w(0)
                    ld("x", 1, 0, HCOL)
                    ld("x", 2, 0, HCOL)
                    ld_w(1)
                    ld("x", 3, 0, HCOL)
                    ld("x", 4, 0, HCOL)
                    ld_w(2)
                    for i in range(5, NHB):
                        ld("x", i, 0, HCOL)
                    nc.sync.dma_start(out=bias_sb, in_=bias_d.ap())
                    for kind, i in [("x", 0), ("0", 0), ("0", 1), ("0", 2),
                                    ("0", 3), ("0", 4), ("x", 1), ("x", 2),
                                    ("x", 3), ("x", 4), ("x", 5), ("x", 6),
                                    ("x", 7)]:
                        ld(kind, i, HCOL, COLS)
                    return
                if boot_split:
                    # group-0 operands at quarter granularity so the first
                    # half-width DVE op can start ~2.5us earlier
                    for kind, i in [("0", 0), ("x", 1), ("x", 2)]:
                        ld(kind, i, 0, QC)
                    ld_w(0)
                    for kind, i in [("x", 3), ("x", 4), ("x", 0)]:
                        ld(kind, i, 0, QC)
                    for kind, i in [("0", 0), ("x", 1), ("x", 2), ("x", 3),
                                    ("x", 4), ("x", 0)]:
                        ld(kind, i, QC, HCOL)
                    for kind, i in [("x", 5), ("x", 6), ("x", 7), ("0", 1)]:
                        ld(kind, i, 0, HCOL)
                    ld_w(1)
                    ld("0", 2, 0, HCOL)
                    ld_w(2)
                    ld("0", 3, 0, HCOL)
                    ld("0", 4, 0, HCOL)
                    nc.sync.dma_start(out=bias_sb, in_=bias_d.ap())
                    for kind, i in use_order:
                        if kind != "w":
                            ld(kind, i, HCOL, COLS)
                    return
                for half, (c0, c1) in enumerate([(0, HCOL), (HCOL, COLS)]):
                    for kind, i in use_order:
                        if kind == "w":
                            if half == 0:
                                ld_w(i)
                        else:
                            ld(kind, i, c0, c1)
                    if half == 0:
                        nc.sync.dma_start(out=bias_sb, in_=bias_d.ap())

            if not full:
                emit_prologue()

            loop_ctx = (
                tc.For_i(
                    0, reps, 1,
                    hint_engines=(mybir.EngineType.PE,),
                    staggered_reset=True,
                )
                if reps > 1
                else contextlib.nullcontext()
            )
            with loop_ctx:
                if full:
                    emit_prologue()
                if psum_pair:
                    # bank-pair PSUM tiles: evictions/stores run as 4 double
                    # width ops (fewer PSUM-read / SBUF-write contention
                    # windows, 4KB store descriptors)
                    pstiles = [
                        ppool.tile([128, 2 * GW], f32, name=f"psp{q}",
                                   tag=f"psp{q}")
                        for q in range(NG // 2)
                    ]
                    psums = [
                        pstiles[q // 2][:, (q % 2) * GW:(q % 2 + 1) * GW]
                        for q in range(NG)
                    ]
                else:
                    psums = [
                        ppool.tile([128, GW], f32, name=f"ps{q}", tag=f"ps{q}")
                        for q in range(NG)
                    ]

                if (reps == 1 or full) and (n_warm or n_fill):
                    # PE clock-ramp warmup/filler scratch matmuls.  DEFAULT
                    # OFF: interleaved HW A/B measures every scratch MM as a
                    # net LOSS (w0f0 100.5us vs w20f5 104.3us single-shot) --
                    # the in-order PE queue delays real MMs behind them and
                    # their SBUF reads contend; the cost model's 2x p-state
                    # ramp penalty does not show up on real HW here.
                    scratch = cpool.tile([128, GW], bf16)
                    nc.scalar.memzero(scratch)
                    for _ in range(n_warm):
                        nc.tensor.matmul(
                            psums[7], lhsT=scratch[:, :128], rhs=scratch,
                            start=True, stop=True,
                        )

                ndve = 0
                npool = 0
                passes = [(0, HCOL, (0, 1, 2, 3)), (HCOL, COLS, (4, 5, 6, 7))]
                # jmaj (non-default, measured ~8us SLOWER single-shot even
                # with w3 repacked in j-major use order): group_list kept
                # only for the experiment record.  NOTE: jmaj=True requires
                # _prep_host to pack w3 as k=j*NMG+g; the default host
                # layout is row-major k=g*NHB+j, so jmaj decode below uses
                # divmod(k, NMG) against a j-major w3 — do not enable
                # without repacking.
                group_list = [
                    (None, list(range(j * NMG, (j + 1) * NMG)))
                    for j in range(NHB)
                ]
                for pi, (c0, c1, banks) in enumerate(passes):
                    width = c1 - c0
                    for gi, (pool_k, pairs) in enumerate(GROUPS):
                        if jmaj:
                            jm_ks = group_list[gi][1]
                            if (boot_split and pi == 0 and gi == 0
                                    and (reps == 1 or full)):
                                # boot group A (j=0, all g): lo/hi half-width
                                # singles, consumable on quarter loads
                                QC = width // 2
                                for half in (0, 1):
                                    cc0 = c0 + half * QC
                                    hentries = []
                                    for ui, k in enumerate(jm_ks):
                                        j2, g2 = divmod(k, NMG)
                                        t = wpool.tile(
                                            [128, QC], bf16,
                                            name=f"jbq{half}_{ui}",
                                            tag=f"jbq{ui}", bufs=1,
                                        )
                                        nc.vector.tensor_mul(
                                            t, xkreps[j2][:, cc0:cc0 + QC],
                                            x0bcs[g2][:, cc0:cc0 + QC],
                                        )
                                        hentries.append((k, t, 0))
                                    for n, (k, t, off) in enumerate(hentries):
                                        for qi2 in range(2):
                                            qb = banks[half * 2 + qi2]
                                            nc.tensor.matmul(
                                                psums[qb],
                                                lhsT=w3_sb[:, k * C:
                                                           (k + 1) * C],
                                                rhs=t[:, off + qi2 * GW:
                                                      off + (qi2 + 1) * GW],
                                                start=(n == 0),
                                                stop=False,
                                            )
                                for _ in range(n_fill):
                                    nc.tensor.matmul(
                                        psums[7], lhsT=scratch[:, :128],
                                        rhs=scratch, start=True, stop=True,
                                    )
                                continue
                            entries = []
                            for k in jm_ks:
                                j2, g2 = divmod(k, NMG)
                                t = wpool.tile(
                                    [128, width], bf16,
                                    name=f"jod{pi}_{gi}_{k}",
                                    tag=f"od1_{ndve % 5}", bufs=1,
                                )
                                ndve += 1
                                nc.vector.tensor_mul(
                                    t, xkreps[j2][:, c0:c1],
                                    x0bcs[g2][:, c0:c1],
                                )
                                entries.append((k, t, 0))
                            ne = len(entries)
                            for n, (k, t, off) in enumerate(entries):
                                for qi, qb in enumerate(banks):
                                    nc.tensor.matmul(
                                        psums[qb],
                                        lhsT=w3_sb[:, k * C:(k + 1) * C],
                                        rhs=t[:, off + qi * GW:
                                              off + (qi + 1) * GW],
                                        start=(gi == 0 and n == 0),
                                        stop=(gi == NGRP - 1 and n == ne - 1),
                                    )
                            if (reps == 1 or full) and pi == 0 and gi < 6:
                                for _ in range(n_fill):
                                    nc.tensor.matmul(
                                        psums[7], lhsT=scratch[:, :128],
                                        rhs=scratch, start=True, stop=True,
                                    )
                            continue
                        if (boot_split and pi == 0 and gi == 0
                                and (reps == 1 or full)):
                            # boot group: two half-width sub-groups (banks
                            # 0-1 then 2-3) so compute starts on quarter
                            # loads; tags reused lo->hi (WAR is benign --
                            # the hi operands arrive later anyway)
                            QC = width // 2
                            for half in (0, 1):
                                cc0 = c0 + half * QC
                                hentries = []
                                for ui, (ka, kb) in enumerate(pairs):
                                    g2, j2 = divmod(ka, NHB)
                                    t = wpool.tile(
                                        [128, 2 * QC], bf16,
                                        name=f"bqp{half}_{ui}",
                                        tag=f"bqp{ui}", bufs=1,
                                    )
                                    nc.vector.tensor_mul(
                                        t.rearrange("p (u c) -> p u c", u=2),
                                        xk3[:, j2:j2 + 2, cc0:cc0 + QC],
                                        x0bcs[g2][:, cc0:cc0 + QC]
                                        .unsqueeze(1)
                                        .broadcast_to([128, 2, QC]),
                                    )
                                    hentries.append((ka, t, 0))
                                    hentries.append((kb, t, QC))
                                g2, j2 = divmod(pool_k, NHB)
                                ts = wpool.tile(
                                    [128, QC], bf16, name=f"bqs{half}",
                                    tag="bqs", bufs=1,
                                )
                                nc.vector.tensor_mul(
                                    ts, xkreps[j2][:, cc0:cc0 + QC],
                                    x0bcs[g2][:, cc0:cc0 + QC],
                                )
                                hentries.append((pool_k, ts, 0))
                                for n, (k, t, off) in enumerate(hentries):
                                    for qi2 in range(2):
                                        qb = banks[half * 2 + qi2]
                                        nc.tensor.matmul(
                                            psums[qb],
                                            lhsT=w3_sb[:, k * C:(k + 1) * C],
                                            rhs=t[:, off + qi2 * GW:
                                                  off + (qi2 + 1) * GW],
                                            start=(n == 0),
                                            stop=False,
                                        )
                            if reps == 1 or full:
                                for _ in range(n_fill):
                                    nc.tensor.matmul(
                                        psums[7], lhsT=scratch[:, :128],
                                        rhs=scratch, start=True, stop=True,
                                    )
                            continue
                        entries = []
                        # Pool chunk first so the slow engine's stream is
                        # maximally early; 3 rotating bufs let it run ahead.
                        if use_pool or pool_noise:
                            g, j = divmod(pool_k, NHB)
                            po = wpool.tile(
                                [128, width], bf16, name=f"po{pi}_{gi}",
                                tag=f"po{npool % N_POOL_BUF}", bufs=1,
                            )
                            npool += 1
                            nc.gpsimd.tensor_mul(
                                po, xkreps[j][:, c0:c1], x0bcs[g][:, c0:c1]
                            )
                        dve_ks = []
                        for ka, kb in pairs:
                            dve_ks.extend([ka, kb])
                        if not use_pool:
                            dve_ks.append(pool_k)
                            dve_ks.sort()
                        if pair:
                            # greedy runs of adjacent same-g chunks, up to
                            # max_pair wide: one DVE op per run with the x0
                            # slice broadcast along the run dim
                            units = []
                            i = 0
                            while i < len(dve_ks):
                                k = dve_ks[i]
                                run = 1
                                while (run < max_pair
                                       and i + run < len(dve_ks)
                                       and dve_ks[i + run] == k + run
                                       and (k + run) % NHB != 0):
                                    run += 1
                                units.append((k, run))
                                i += run
                        else:
                            units = [(k, 1) for k in dve_ks]
                        for k, nun in units:
                            g2, j2 = divmod(k, NHB)
                            nbuf = (N_DVE_BUF if nun == 2
                                    else (4 if nun == 1 else 3))
                            t = wpool.tile(
                                [128, nun * width], bf16,
                                name=f"od{pi}_{gi}_{k}",
                                tag=f"od{nun}_{ndve % nbuf}", bufs=1,
                            )
                            ndve += 1
                            if nun > 1:
                                nc.vector.tensor_mul(
                                    t.rearrange("p (u c) -> p u c", u=nun),
                                    xk3[:, j2:j2 + nun, c0:c1],
                                    x0bcs[g2][:, c0:c1]
                                    .unsqueeze(1)
                                    .broadcast_to([128, nun, width]),
                                )
                                for u in range(nun):
                                    entries.append((k + u, t, u * width))
                            else:
                                nc.vector.tensor_mul(
                                    t, xkreps[j2][:, c0:c1],
                                    x0bcs[g2][:, c0:c1],
                                )
                                entries.append((k, t, 0))
                        # Pool chunk consumed last: it was produced with ~1
                        # group of lag.  Chunk-major MM order (all banks per
                        # chunk) so the PE consumes each chunk the moment it
                        # is produced instead of stalling a whole bank sweep
                        # on the last chunk of the group; also reuses the
                        # stationary w3 chunk across the 4 bank MMs.
                        if use_pool:
                            entries.append((pool_k, po, 0))
                        ne = len(entries)
                        if mm_order == "chunk":
                            order = [(n, qi) for n in range(ne)
                                     for qi in range(len(banks))]
                        else:
                            order = [(n, qi) for qi in range(len(banks))
                                     for n in range(ne)]
                        for n, qi in order:
                            k, t, off = entries[n]
                            qb = banks[qi]
                            nc.tensor.matmul(
                                psums[qb],
                                lhsT=w3_sb[:, k * C:(k + 1) * C],
                                rhs=t[:, off + qi * GW:
                                      off + (qi + 1) * GW],
                                start=(gi == 0 and n == 0),
                                stop=(not (direct_store or bias_mm)
                                      and gi == NGRP - 1 and n == ne - 1),
                            )
                        if (reps == 1 or full) and pi == 0 and gi < 6:
                            # bootstrap filler: the prologue DMA stream can
                            # momentarily starve the PE here; dep-free
                            # scratch matmuls into the not-yet-active bank 7
                            # absorb the stall (an idle gap resets the clock
                            # ramp, costing ~2x on the next ~3us of MMs)
                            for _ in range(n_fill):
                                nc.tensor.matmul(
                                    psums[7], lhsT=scratch[:, :128],
                                    rhs=scratch, start=True, stop=True,
                                )
                    if skip_out:
                        pass
                    elif direct_store:
                        # bias folded into a K=1 matmul (the stop MM of each
                        # bank), then DMA the PSUM bank straight to DRAM:
                        # no ScalarE eviction, no SBUF store traffic.
                        for qi, qb in enumerate(banks):
                            nc.tensor.matmul(
                                psums[qb],
                                lhsT=biasr_sb[0:1, :],
                                rhs=ones_sb[0:1, :],
                                start=False,
                                stop=True,
                            )
                            nc.scalar.dma_start(
                                out=out_d.ap()[:, qb * bpg:(qb + 1) * bpg, :],
                                in_=psums[qb],
                            )
                    else:
                        # bias-add eviction per bank on ScalarE; stores ride
                        # the ScalarE DGE queue so they never queue behind
                        # loads.
                        store_eng = nc.scalar if store_q == "scalar" else nc.sync
                        if psum_pair and not bias_mm and evict_split == 1:
                            for qp in range(len(banks) // 2):
                                qb = banks[2 * qp]
                                out_sb = opool.tile(
                                    [128, 2 * GW], f32, name=f"osbp{pi}{qb}",
                                    tag=f"osbp{qp}",
                                )
                                nc.scalar.activation(
                                    out_sb,
                                    pstiles[qb // 2],
                                    mybir.ActivationFunctionType.Identity,
                                    bias=bias_sb[:, 0:1],
                                    scale=1.0,
                                )
                                if not skip_store:
                                    store_eng.dma_start(
                                        out=out_d.ap()[:, qb * bpg:
                                                       (qb + 2) * bpg, :],
                                        in_=out_sb,
                                    )
                            continue_evict = False
                        else:
                            continue_evict = True
                        out_dt = bf16 if out_bf16 else f32
                        # single-shot tail: the DVE is idle once its last op
                        # retires, so split the final pass's evictions
                        # between ScalarE and DVE and its stores across both
                        # DGE queues -- halves the post-last-matmul chain
                        tail_mode = (tail_par and (reps == 1 or full)
                                     and pi == len(passes) - 1)
                        for qi, qb in enumerate(banks):
                            if not continue_evict:
                                break
                            if bias_mm:
                                # fold bias into the bank's stop MM (K=1)
                                nc.tensor.matmul(
                                    psums[qb],
                                    lhsT=biasr_sb[0:1, :],
                                    rhs=ones_sb[0:1, :],
                                    start=False,
                                    stop=True,
                                )
                            out_sb = opool.tile(
                                [128, GW], out_dt, name=f"osb{pi}{qb}",
                                tag=f"osb{qi}",
                            )
                            if tail_mode and qi % 2 == 1:
                                nc.vector.tensor_scalar_add(
                                    out_sb, psums[qb], bias_sb[:, 0:1]
                                )
                                nc.scalar.dma_start(
                                    out=out_d.ap()[:, qb * bpg:
                                                   (qb + 1) * bpg, :],
                                    in_=out_sb,
                                )
                                continue
                            es = GW // evict_split
                            for v in range(evict_split):
                                if bias_mm:
                                    nc.scalar.activation(
                                        out_sb[:, v * es:(v + 1) * es],
                                        psums[qb][:, v * es:(v + 1) * es],
                                        mybir.ActivationFunctionType.Copy,
                                    )
                                else:
                                    nc.scalar.activation(
                                        out_sb[:, v * es:(v + 1) * es],
                                        psums[qb][:, v * es:(v + 1) * es],
                                        mybir.ActivationFunctionType.Identity,
                                        bias=bias_sb[:, 0:1],
                                        scale=1.0,
                                    )
                            if not skip_store:
                                store_eng.dma_start(
                                    out=out_d.ap()[:, qb * bpg:(qb + 1) * bpg, :],
                                    in_=out_sb,
                                )

    nc.compile()
    return nc


def _prep_host(x0, xk, W, b):
    """Host-side layout prep (no arithmetic): shard, transpose, replicate."""
    part = np.arange(128)
    hh = (part % HB)[None, :] + HB * np.arange(NHB)[:, None]   # [NHB, 128]
    mm = (part // HB)[None, :] + MG * np.arange(NMG)[:, None]  # [NMG, 128]

    Wr = W.reshape(C, H, M)
    w3 = np.empty((128, NCHUNK, C), ml_dtypes.bfloat16)
    for g in range(NMG):
        for j in range(NHB):
            w3[:, g * NHB + j, :] = Wr[:, hh[j], mm[g]].T.astype(
                ml_dtypes.bfloat16
            )
    w3 = np.ascontiguousarray(w3.reshape(128, NCHUNK * C))
    bias = np.ascontiguousarray(b.reshape(C, 1)).astype(np.float32)

    in_maps = []
    for k in range(N_CORES):
        x0s = x0[k * BC:(k + 1) * BC]            # [BC, M, D]
        xks = xk[k * BC:(k + 1) * BC]            # [BC, H, D]
        xk2 = (
            np.ascontiguousarray(xks.transpose(1, 0, 2))
            .reshape(H, COLS)
            .astype(ml_dtypes.bfloat16)
        )
        x02 = (
            np.ascontiguousarray(x0s.transpose(1, 0, 2))
            .reshape(M, COLS)
            .astype(ml_dtypes.bfloat16)
        )
        in_maps.append(
            {
                "xkrep_in": np.ascontiguousarray(xk2[hh]),
                "x0bc_in": np.ascontiguousarray(x02[mm]),
                "w3_in": w3,
                "bias_in": bias,
                "biasr_in": np.ascontiguousarray(
                    b.reshape(1, C)
                ).astype(ml_dtypes.bfloat16),
            }
        )
    return in_maps


def _run(in_maps, **kwargs):
    from concourse import bass_utils

    if "nc" not in _cache:
        _cache["nc"] = _build()
    return bass_utils.run_bass_kernel_spmd(
        _cache["nc"], in_maps, core_ids=list(range(N_CORES)), **kwargs
    )


def kernel(x0, xk, W, b, _bench=[None]):
    x0 = np.asarray(x0, dtype=np.float32)
    xk = np.asarray(xk, dtype=np.float32)
    W = np.asarray(W, dtype=np.float32)
    b = np.asarray(b, dtype=np.float32)
    in_maps = _prep_host(x0, xk, W, b)
    res = _run(in_maps)
    _bench[0] = res
    # per-core out is c-major [C, BC, D] (bf16); restore [BC, C, D], stack
    # cores, upcast to f32 on host
    out = np.concatenate(
        [np.transpose(np.asarray(r["out"], dtype=np.float32), (1, 0, 2))
         for r in res.results],
        axis=0,
    )
    return np.ascontiguousarray(out, dtype=np.float32)


# revision 86
# speedup vs baseline: 1.0348x; 1.0333x over previous
"""Trainium2 Bass kernel for CIN layer:
    out[b,c,d] = sum_{h,m} W[c, h*M+m] * xk[b,h,d] * x0[b,m,d] + bias[c]

Shapes (hardcoded): x0 [512,40,64] f32, xk [512,128,64] f32,
W [128,5120] f32, b [128] f32 -> out [512,128,64] f32.

Strategy: data-parallel over batch B across 8 cores (64 batches/core).
Per core, columns are the 64*64=4096 (b,d) pairs.  The 5120-long (h,m)
contraction is split into 40 chunks of 128 rows with a mixed-radix
partition layout: chunk k=(g*8+j) covers m in the 8-wide group g (5
groups) x h in the 16-wide block j (8 blocks); partition p holds
(m = 8g + p//16, h = 16j + p%16).  Then per chunk
  outer[p, col] = xkrep_j[p, col] * x0bc_g[p, col]   (elementwise bf16)
  psum[bank]   += w3[k][p,c].T @ outer[:, bank*512:] (PE, 40-chunk accum)
xkrep_j / x0bc_g are replicated host-side (pure layout, no math).

Engine balance (HW-measured): the 21M-elem/core outer-product stream
is produced ENTIRELY on the DVE (TensorTensor bf16 2x mode: 2 elem/
lane/cycle at 0.96 GHz = 245.8 Gel/s -> ~89us/iter), adjacent same-g
chunk pairs fused into one double-width op with the shared x0 slice
broadcast along the middle dim (saves the per-op ramp overhead).  The
PE only needs 68.3us for its 320 matmuls, so it trails the DVE.
IMPORTANT NEGATIVE RESULT: offloading chunks to GpSimd/Pool (62.8
Gel/s solo) is a large net LOSS in situ -- even dep-free dummy Pool
TensorTensors alongside the DVE+PE stream blow per-iter time from
~94us to ~159us (SBUF bandwidth contention).  Keep Pool idle.

Column passes: two half-width passes, A=[0,2048) into PSUM banks 0-3,
B=[2048,4096) into banks 4-7.  MM emission is chunk-major (all 4 bank
MMs per chunk back-to-back) so the PE consumes each outer tile the
moment the DVE finishes it and the tile's buffer frees quickly --
bank-major sweeps hold buffers a whole group and stall the DVE on
tag-rotation WAR (~+9us/pass, measured).  The prologue DMA stream
(14.9MB, ~45us at 332 GB/s) is issued at half-column granularity in
first-use order on the sync queue, so pass A's operands land in the
first ~7us while pass A runs ~45us.  NO warmup/filler scratch matmuls:
the cost model's PE clock-ramp penalty (idle gap -> 1.2 GHz for ~3us)
does NOT materialize on real HW -- interleaved A/B measures every
scratch MM as a net loss (~3.8us single-shot for 50 of them), since
the in-order PE queue delays real MMs behind them and their SBUF
reads contend.  Groups 0-1 of pass A run as two
half-width sub-groups each (banks 0-1 then 2-3) fed by quarter-
granularity loads, so the first DVE op starts ~2.5us earlier and the
group-1 operand wait shrinks (~1.6us combined, interleaved A/B).  Pass eviction
(ScalarE bias-add) is per-bank; stores go on the SYNC DGE queue --
putting them on the ScalarE queue delays the eviction chain by its
667ns/DMA config time and costs ~2us/iter (measured).  The remaining
~3us/iter of eviction overhead is a fixed sync/chain latency, NOT
contention: 256-wide, 1024-wide (bank-pair PSUM tiles), bias-via-K=1-
matmul + Copy, and bf16 eviction+store (half the bytes, host upcast)
variants ALL measured equal or slower than plain 512-wide f32
Identity+bias evicts.  Output overhead overall: ~89us/iter with no
output path (exactly the DVE roofline), ~93us with it.  Single-shot
tail: the final pass's evictions alternate ScalarE / DVE
(tensor_scalar_add carries the bias) and its stores alternate the two
DGE queues -- the DVE is idle once its last outer op retires, so this
halves the post-last-matmul chain (~0.6us, measured).

DMA-descriptor shaping: W is pre-transposed to [128, 40*128] (10KB
contiguous per partition) and the output DRAM tensor is c-major
[C, BC, D] (2KB contiguous runs); descriptors under 512B pay a 2x
DMA-time penalty.  The reps>1 build (used for steady-state timing)
is the same two-pass body inside a For_i hardware loop.
"""

import numpy as np
import ml_dtypes

B, M, H, D, C = 512, 40, 128, 64, 128
N_CORES = 8
BC = B // N_CORES          # 64 batches per core
COLS = BC * D              # 4096 (b,d) columns per core
NG = 8                     # PSUM banks
GW = COLS // NG            # 512 columns per bank
MG = 8                     # m-values per chunk group
NMG = M // MG              # 5 m-groups
HB = 128 // MG             # 16 h-values per block
NHB = H // HB              # 8 h-blocks
NCHUNK = NMG * NHB         # 40 contraction chunks

_cache = {}


def _build(reps=1, n_warm=0, n_fill=0, mm_order="chunk", use_pool=False,
           pair=True, pool_noise=False, max_pair=2, direct_store=False,
           full=False, dma2q=False, skip_out=False, skip_store=False,
           ndve_buf=5, evict_split=1, store_q="sync", bias_mm=False,
           boot_split=True, psum_pair=False, out_bf16=False, tail_par=True,
           jmaj=False, boot_grps=2):
    import contextlib

    import concourse.bacc as bacc
    import concourse.mybir as mybir
    from concourse.tile import TileContext

    f32 = mybir.dt.float32
    bf16 = mybir.dt.bfloat16

    nc = bacc.Bacc("TRN2", debug=False, num_devices=N_CORES)

    xkr_d = nc.dram_tensor("xkrep_in", [NHB, 128, COLS], bf16, kind="ExternalInput")
    x0b_d = nc.dram_tensor("x0bc_in", [NMG, 128, COLS], bf16, kind="ExternalInput")
    # pre-transposed: partition-major, 10KB contiguous per partition row
    w3_d = nc.dram_tensor("w3_in", [128, NCHUNK * C], bf16, kind="ExternalInput")
    bias_d = nc.dram_tensor("bias_in", [C, 1], f32, kind="ExternalInput")
    # bias as a single-partition row for the K=1 bias matmul (direct_store)
    biasr_d = nc.dram_tensor("biasr_in", [1, C], bf16, kind="ExternalInput")
    # c-major so each output descriptor is a contiguous (b,d) run; bf16
    # halves the evict/store traffic (host upcasts to f32 after gather)
    out_d = nc.dram_tensor("out", [C, BC, D], bf16 if out_bf16 else f32,
                           kind="ExternalOutput")

    GK = 5
    NGRP = NCHUNK // GK        # 8 groups per pass
    HCOL = COLS // 2           # 2048
    bpg = BC // NG             # 8 batches per bank

    # Per 5-chunk group: one chunk to Pool, remaining four as two adjacent
    # (same-g, j/j+1) DVE pairs.  Chosen so every group pairs cleanly.
    POOL_CHUNKS = (0, 7, 10, 15, 24, 25, 34, 35)
    GROUPS = []
    for gi in range(NGRP):
        ks = list(range(gi * GK, gi * GK + GK))
        pk = [k for k in ks if k in POOL_CHUNKS]
        assert len(pk) == 1
        rest = [k for k in ks if k != pk[0]]
        pairs = [(rest[0], rest[1]), (rest[2], rest[3])]
        for a, b in pairs:
            assert b == a + 1 and a % NHB != NHB - 1 and a // NHB == b // NHB
        GROUPS.append((pk[0], pairs))

    N_DVE_BUF = ndve_buf
    N_POOL_BUF = 3

    with TileContext(nc) as tc:
        with (
            tc.tile_pool(name="const", bufs=1) as cpool,
            tc.tile_pool(name="work", bufs=1) as wpool,
            tc.tile_pool(name="outp", bufs=1) as opool,
            tc.tile_pool(name="psum", bufs=1, space="PSUM") as ppool,
        ):
            # ---- SBUF constant tiles ----
            w3_sb = cpool.tile([128, NCHUNK * C], bf16)
            bias_sb = cpool.tile([128, 1], f32)
            biasr_sb = cpool.tile([1, C], bf16, name="biasr")
            ones_sb = cpool.tile([1, GW], bf16, name="ones1")
            xkall = cpool.tile([128, NHB * COLS], bf16, name="xkall")
            xkreps = [xkall[:, i * COLS:(i + 1) * COLS] for i in range(NHB)]
            xk3 = xkall.rearrange("p (j c) -> p j c", c=COLS)
            x0bcs = [
                cpool.tile([128, COLS], bf16, name=f"x0b{i}", tag=f"x0b{i}")
                for i in range(NMG)
            ]

            # ---- prologue DMA: half-column granularity, first-use order ---
            # Single (sync-queue) stream at full DMA bandwidth.  w3 rides in
            # three slices ordered by the chunk ranges that consume them;
            # bias is only needed by the first eviction (~40us in).
            _ldn = [0]

            def ld(kind, i, c0, c1):
                t = xkreps[i] if kind == "x" else x0bcs[i]
                src = (xkr_d if kind == "x" else x0b_d).ap()[i]
                # two DGE queues so two DMA engines stream concurrently
                eng = nc.sync if (not dma2q or _ldn[0] % 2 == 0) else nc.scalar
                _ldn[0] += 1
                eng.dma_start(out=t[:, c0:c1], in_=src[:, c0:c1])

            use_order = [("0", 0), ("x", 0), ("x", 1), ("w", 0), ("x", 2),
                         ("x", 3), ("x", 4), ("x", 5), ("x", 6), ("x", 7),
                         ("0", 1), ("w", 1), ("0", 2), ("w", 2), ("0", 3),
                         ("0", 4)]
            w_slices = [(0, 10 * C), (10 * C, 25 * C), (25 * C, NCHUNK * C)]
            nc.vector.memset(ones_sb, 1.0)
            nc.sync.dma_start(out=biasr_sb, in_=biasr_d.ap())

            def ld_w(i):
                s0, s1 = w_slices[i]
                nc.sync.dma_start(out=w3_sb[:, s0:s1], in_=w3_d.ap()[:, s0:s1])

            def emit_prologue():
                QC = HCOL // 2
                if jmaj:
                    # j-major consumption: after group A (xj0 + all five x0
                    # tiles), each group needs ONE new xj half (1.54us of
                    # DMA) vs 5.45us of DVE work -- the DMA never paces the
                    # DVE past ~14us.  w3 loads after the x0 tiles; the PE
                    # trails the DVE anyway, so its later start is free.
                    if boot_split:
                        for kind, i in [("x", 0), ("0", 0), ("0", 1),
                                        ("0", 2), ("0", 3), ("0", 4)]:
                            ld(kind, i, 0, QC)
                        for kind, i in [("x", 0), ("0", 0), ("0", 1),
                                        ("0", 2), ("0", 3), ("0", 4)]:
                            ld(kind, i, QC, HCOL)
                    else:
                        for kind, i in [("x", 0), ("0", 0), ("0", 1),
                                        ("0", 2), ("0", 3), ("0", 4)]:
                            ld(kind, i, 0, HCOL)
                    # w3 k'-order slices: [0:10C)=j0-j1, [10C:25C)=j2-j4,
                    # rest -- each lands just ahead of its consuming groups
                    ld_w(0)
                    ld("x", 1, 0, HCOL)
                    ld("x", 2, 0, HCOL)
                    ld_w(1)
                    ld("x", 3, 0, HCOL)
                    ld("x", 4, 0, HCOL)
                    ld_w(2)
                    for i in range(5, NHB):
                        ld("x", i, 0, HCOL)
                    nc.sync.dma_start(out=bias_sb, in_=bias_d.ap())
                    for kind, i in [("x", 0), ("0", 0), ("0", 1), ("0", 2),
                                    ("0", 3), ("0", 4), ("x", 1), ("x", 2),
                                    ("x", 3), ("x", 4), ("x", 5), ("x", 6),
                                    ("x", 7)]:
                        ld(kind, i, HCOL, COLS)
                    return
                if boot_split:
                    # group-0 operands at quarter granularity so the first
                    # half-width DVE op can start ~2.5us earlier
                    for kind, i in [("0", 0), ("x", 1), ("x", 2)]:
                        ld(kind, i, 0, QC)
                    ld_w(0)
                    for kind, i in [("x", 3), ("x", 4), ("x", 0)]:
                        ld(kind, i, 0, QC)
                    for kind, i in [("0", 0), ("x", 1), ("x", 2), ("x", 3),
                                    ("x", 4), ("x", 0)]:
                        ld(kind, i, QC, HCOL)
                    if boot_grps >= 2:
                        # group-1 operands also at quarter granularity
                        for kind, i in [("x", 5), ("x", 6), ("x", 7),
                                        ("0", 1)]:
                            ld(kind, i, 0, QC)
                        for kind, i in [("x", 5), ("x", 6), ("x", 7),
                                        ("0", 1)]:
                            ld(kind, i, QC, HCOL)
                    else:
                        for kind, i in [("x", 5), ("x", 6), ("x", 7),
                                        ("0", 1)]:
                            ld(kind, i, 0, HCOL)
                    ld_w(1)
                    ld("0", 2, 0, HCOL)
                    ld_w(2)
                    ld("0", 3, 0, HCOL)
                    ld("0", 4, 0, HCOL)
                    nc.sync.dma_start(out=bias_sb, in_=bias_d.ap())
                    for kind, i in use_order:
                        if kind != "w":
                            ld(kind, i, HCOL, COLS)
                    return
                for half, (c0, c1) in enumerate([(0, HCOL), (HCOL, COLS)]):
                    for kind, i in use_order:
                        if kind == "w":
                            if half == 0:
                                ld_w(i)
                        else:
                            ld(kind, i, c0, c1)
                    if half == 0:
                        nc.sync.dma_start(out=bias_sb, in_=bias_d.ap())

            if not full:
                emit_prologue()

            loop_ctx = (
                tc.For_i(
                    0, reps, 1,
                    hint_engines=(mybir.EngineType.PE,),
                    staggered_reset=True,
                )
                if reps > 1
                else contextlib.nullcontext()
            )
            with loop_ctx:
                if full:
                    emit_prologue()
                if psum_pair:
                    # bank-pair PSUM tiles: evictions/stores run as 4 double
                    # width ops (fewer PSUM-read / SBUF-write contention
                    # windows, 4KB store descriptors)
                    pstiles = [
                        ppool.tile([128, 2 * GW], f32, name=f"psp{q}",
                                   tag=f"psp{q}")
                        for q in range(NG // 2)
                    ]
                    psums = [
                        pstiles[q // 2][:, (q % 2) * GW:(q % 2 + 1) * GW]
                        for q in range(NG)
                    ]
                else:
                    psums = [
                        ppool.tile([128, GW], f32, name=f"ps{q}", tag=f"ps{q}")
                        for q in range(NG)
                    ]

                if (reps == 1 or full) and (n_warm or n_fill):
                    # PE clock-ramp warmup/filler scratch matmuls.  DEFAULT
                    # OFF: interleaved HW A/B measures every scratch MM as a
                    # net LOSS (w0f0 100.5us vs w20f5 104.3us single-shot) --
                    # the in-order PE queue delays real MMs behind them and
                    # their SBUF reads contend; the cost model's 2x p-state
                    # ramp penalty does not show up on real HW here.
                    scratch = cpool.tile([128, GW], bf16)
                    nc.scalar.memzero(scratch)
                    for _ in range(n_warm):
                        nc.tensor.matmul(
                            psums[7], lhsT=scratch[:, :128], rhs=scratch,
                            start=True, stop=True,
                        )

                ndve = 0
                npool = 0
                passes = [(0, HCOL, (0, 1, 2, 3)), (HCOL, COLS, (4, 5, 6, 7))]
                # jmaj (non-default, measured ~8us SLOWER single-shot even
                # with w3 repacked in j-major use order): group_list kept
                # only for the experiment record.  NOTE: jmaj=True requires
                # _prep_host to pack w3 as k=j*NMG+g; the default host
                # layout is row-major k=g*NHB+j, so jmaj decode below uses
                # divmod(k, NMG) against a j-major w3 — do not enable
                # without repacking.
                group_list = [
                    (None, list(range(j * NMG, (j + 1) * NMG)))
                    for j in range(NHB)
                ]
                for pi, (c0, c1, banks) in enumerate(passes):
                    width = c1 - c0
                    for gi, (pool_k, pairs) in enumerate(GROUPS):
                        if jmaj:
                            jm_ks = group_list[gi][1]
                            if (boot_split and pi == 0 and gi == 0
                                    and (reps == 1 or full)):
                                # boot group A (j=0, all g): lo/hi half-width
                                # singles, consumable on quarter loads
                                QC = width // 2
                                for half in (0, 1):
                                    cc0 = c0 + half * QC
                                    hentries = []
                                    for ui, k in enumerate(jm_ks):
                                        j2, g2 = divmod(k, NMG)
                                        t = wpool.tile(
                                            [128, QC], bf16,
                                            name=f"jbq{half}_{ui}",
                                            tag=f"jbq{ui}", bufs=1,
                                        )
                                        nc.vector.tensor_mul(
                                            t, xkreps[j2][:, cc0:cc0 + QC],
                                            x0bcs[g2][:, cc0:cc0 + QC],
                                        )
                                        hentries.append((k, t, 0))
                                    for n, (k, t, off) in enumerate(hentries):
                                        for qi2 in range(2):
                                            qb = banks[half * 2 + qi2]
                                            nc.tensor.matmul(
                                                psums[qb],
                                                lhsT=w3_sb[:, k * C:
                                                           (k + 1) * C],
                                                rhs=t[:, off + qi2 * GW:
                                                      off + (qi2 + 1) * GW],
                                                start=(n == 0),
                                                stop=False,
                                            )
                                for _ in range(n_fill):
                                    nc.tensor.matmul(
                                        psums[7], lhsT=scratch[:, :128],
                                        rhs=scratch, start=True, stop=True,
                                    )
                                continue
                            entries = []
                            for k in jm_ks:
                                j2, g2 = divmod(k, NMG)
                                t = wpool.tile(
                                    [128, width], bf16,
                                    name=f"jod{pi}_{gi}_{k}",
                                    tag=f"od1_{ndve % 5}", bufs=1,
                                )
                                ndve += 1
                                nc.vector.tensor_mul(
                                    t, xkreps[j2][:, c0:c1],
                                    x0bcs[g2][:, c0:c1],
                                )
                                entries.append((k, t, 0))
                            ne = len(entries)
                            for n, (k, t, off) in enumerate(entries):
                                for qi, qb in enumerate(banks):
                                    nc.tensor.matmul(
                                        psums[qb],
                                        lhsT=w3_sb[:, k * C:(k + 1) * C],
                                        rhs=t[:, off + qi * GW:
                                              off + (qi + 1) * GW],
                                        start=(gi == 0 and n == 0),
                                        stop=(gi == NGRP - 1 and n == ne - 1),
                                    )
                            if (reps == 1 or full) and pi == 0 and gi < 6:
                                for _ in range(n_fill):
                                    nc.tensor.matmul(
                                        psums[7], lhsT=scratch[:, :128],
                                        rhs=scratch, start=True, stop=True,
                                    )
                            continue
                        if (boot_split and pi == 0 and gi < boot_grps
                                and (reps == 1 or full)):
                            # boot group: two half-width sub-groups (banks
                            # 0-1 then 2-3) so compute starts on quarter
                            # loads; tags reused lo->hi (WAR is benign --
                            # the hi operands arrive later anyway)
                            QC = width // 2
                            for half in (0, 1):
                                cc0 = c0 + half * QC
                                hentries = []
                                for ui, (ka, kb) in enumerate(pairs):
                                    g2, j2 = divmod(ka, NHB)
                                    t = wpool.tile(
                                        [128, 2 * QC], bf16,
                                        name=f"bqp{half}_{ui}",
                                        tag=f"bqp{ui}", bufs=1,
                                    )
                                    nc.vector.tensor_mul(
                                        t.rearrange("p (u c) -> p u c", u=2),
                                        xk3[:, j2:j2 + 2, cc0:cc0 + QC],
                                        x0bcs[g2][:, cc0:cc0 + QC]
                                        .unsqueeze(1)
                                        .broadcast_to([128, 2, QC]),
                                    )
                                    hentries.append((ka, t, 0))
                                    hentries.append((kb, t, QC))
                                g2, j2 = divmod(pool_k, NHB)
                                ts = wpool.tile(
                                    [128, QC], bf16, name=f"bqs{half}",
                                    tag="bqs", bufs=1,
                                )
                                nc.vector.tensor_mul(
                                    ts, xkreps[j2][:, cc0:cc0 + QC],
                                    x0bcs[g2][:, cc0:cc0 + QC],
                                )
                                hentries.append((pool_k, ts, 0))
                                for n, (k, t, off) in enumerate(hentries):
                                    for qi2 in range(2):
                                        qb = banks[half * 2 + qi2]
                                        nc.tensor.matmul(
                                            psums[qb],
                                            lhsT=w3_sb[:, k * C:(k + 1) * C],
                                            rhs=t[:, off + qi2 * GW:
                                                  off + (qi2 + 1) * GW],
                                            start=(gi == 0 and n == 0),
                                            stop=False,
                                        )
                            if reps == 1 or full:
                                for _ in range(n_fill):
                                    nc.tensor.matmul(
                                        psums[7], lhsT=scratch[:, :128],
                                        rhs=scratch, start=True, stop=True,
                                    )
                            continue
                        entries = []
                        # Pool chunk first so the slow engine's stream is
                        # maximally early; 3 rotating bufs let it run ahead.
                        if use_pool or pool_noise:
                            g, j = divmod(pool_k, NHB)
                            po = wpool.tile(
                                [128, width], bf16, name=f"po{pi}_{gi}",
                                tag=f"po{npool % N_POOL_BUF}", bufs=1,
                            )
                            npool += 1
                            nc.gpsimd.tensor_mul(
                                po, xkreps[j][:, c0:c1], x0bcs[g][:, c0:c1]
                            )
                        dve_ks = []
                        for ka, kb in pairs:
                            dve_ks.extend([ka, kb])
                        if not use_pool:
                            dve_ks.append(pool_k)
                            dve_ks.sort()
                        if pair:
                            # greedy runs of adjacent same-g chunks, up to
                            # max_pair wide: one DVE op per run with the x0
                            # slice broadcast along the run dim
                            units = []
                            i = 0
                            while i < len(dve_ks):
                                k = dve_ks[i]
                                run = 1
                                while (run < max_pair
                                       and i + run < len(dve_ks)
                                       and dve_ks[i + run] == k + run
                                       and (k + run) % NHB != 0):
                                    run += 1
                                units.append((k, run))
                                i += run
                        else:
                            units = [(k, 1) for k in dve_ks]
                        for k, nun in units:
                            g2, j2 = divmod(k, NHB)
                            nbuf = (N_DVE_BUF if nun == 2
                                    else (4 if nun == 1 else 3))
                            t = wpool.tile(
                                [128, nun * width], bf16,
                                name=f"od{pi}_{gi}_{k}",
                                tag=f"od{nun}_{ndve % nbuf}", bufs=1,
                            )
                            ndve += 1
                            if nun > 1:
                                nc.vector.tensor_mul(
                                    t.rearrange("p (u c) -> p u c", u=nun),
                                    xk3[:, j2:j2 + nun, c0:c1],
                                    x0bcs[g2][:, c0:c1]
                                    .unsqueeze(1)
                                    .broadcast_to([128, nun, width]),
                                )
                                for u in range(nun):
                                    entries.append((k + u, t, u * width))
                            else:
                                nc.vector.tensor_mul(
                                    t, xkreps[j2][:, c0:c1],
                                    x0bcs[g2][:, c0:c1],
                                )
                                entries.append((k, t, 0))
                        # Pool chunk consumed last: it was produced with ~1
                        # group of lag.  Chunk-major MM order (all banks per
                        # chunk) so the PE consumes each chunk the moment it
                        # is produced instead of stalling a whole bank sweep
                        # on the last chunk of the group; also reuses the
                        # stationary w3 chunk across the 4 bank MMs.
                        if use_pool:
                            entries.append((pool_k, po, 0))
                        ne = len(entries)
                        if mm_order == "chunk":
                            order = [(n, qi) for n in range(ne)
                                     for qi in range(len(banks))]
                        else:
                            order = [(n, qi) for qi in range(len(banks))
                                     for n in range(ne)]
                        for n, qi in order:
                            k, t, off = entries[n]
                            qb = banks[qi]
                            nc.tensor.matmul(
                                psums[qb],
                                lhsT=w3_sb[:, k * C:(k + 1) * C],
                                rhs=t[:, off + qi * GW:
                                      off + (qi + 1) * GW],
                                start=(gi == 0 and n == 0),
                                stop=(not (direct_store or bias_mm)
                                      and gi == NGRP - 1 and n == ne - 1),
                            )
                        if (reps == 1 or full) and pi == 0 and gi < 6:
                            # bootstrap filler: the prologue DMA stream can
                            # momentarily starve the PE here; dep-free
                            # scratch matmuls into the not-yet-active bank 7
                            # absorb the stall (an idle gap resets the clock
                            # ramp, costing ~2x on the next ~3us of MMs)
                            for _ in range(n_fill):
                                nc.tensor.matmul(
                                    psums[7], lhsT=scratch[:, :128],
                                    rhs=scratch, start=True, stop=True,
                                )
                    if skip_out:
                        pass
                    elif direct_store:
                        # bias folded into a K=1 matmul (the stop MM of each
                        # bank), then DMA the PSUM bank straight to DRAM:
                        # no ScalarE eviction, no SBUF store traffic.
                        for qi, qb in enumerate(banks):
                            nc.tensor.matmul(
                                psums[qb],
                                lhsT=biasr_sb[0:1, :],
                                rhs=ones_sb[0:1, :],
                                start=False,
                                stop=True,
                            )
                            nc.scalar.dma_start(
                                out=out_d.ap()[:, qb * bpg:(qb + 1) * bpg, :],
                                in_=psums[qb],
                            )
                    else:
                        # bias-add eviction per bank on ScalarE; stores ride
                        # the ScalarE DGE queue so they never queue behind
                        # loads.
                        store_eng = nc.scalar if store_q == "scalar" else nc.sync
                        if psum_pair and not bias_mm and evict_split == 1:
                            for qp in range(len(banks) // 2):
                                qb = banks[2 * qp]
                                out_sb = opool.tile(
                                    [128, 2 * GW], f32, name=f"osbp{pi}{qb}",
                                    tag=f"osbp{qp}",
                                )
                                nc.scalar.activation(
                                    out_sb,
                                    pstiles[qb // 2],
                                    mybir.ActivationFunctionType.Identity,
                                    bias=bias_sb[:, 0:1],
                                    scale=1.0,
                                )
                                if not skip_store:
                                    store_eng.dma_start(
                                        out=out_d.ap()[:, qb * bpg:
                                                       (qb + 2) * bpg, :],
                                        in_=out_sb,
                                    )
                            continue_evict = False
                        else:
                            continue_evict = True
                        out_dt = bf16 if out_bf16 else f32
                        # single-shot tail: the DVE is idle once its last op
                        # retires, so split the final pass's evictions
                        # between ScalarE and DVE and its stores across both
                        # DGE queues -- halves the post-last-matmul chain
                        tail_mode = (tail_par and (reps == 1 or full)
                                     and pi == len(passes) - 1)
                        for qi, qb in enumerate(banks):
                            if not continue_evict:
                                break
                            if bias_mm:
                                # fold bias into the bank's stop MM (K=1)
                                nc.tensor.matmul(
                                    psums[qb],
                                    lhsT=biasr_sb[0:1, :],
                                    rhs=ones_sb[0:1, :],
                                    start=False,
                                    stop=True,
                                )
                            out_sb = opool.tile(
                                [128, GW], out_dt, name=f"osb{pi}{qb}",
                                tag=f"osb{qi}",
                            )
                            if tail_mode and qi % 2 == 1:
                                nc.vector.tensor_scalar_add(
                                    out_sb, psums[qb], bias_sb[:, 0:1]
                                )
                                nc.scalar.dma_start(
                                    out=out_d.ap()[:, qb * bpg:
                                                   (qb + 1) * bpg, :],
                                    in_=out_sb,
                                )
                                continue
                            es = GW // evict_split
                            for v in range(evict_split):
                                if bias_mm:
                                    nc.scalar.activation(
                                        out_sb[:, v * es:(v + 1) * es],
                                        psums[qb][:, v * es:(v + 1) * es],
                                        mybir.ActivationFunctionType.Copy,
                                    )
                                else:
                                    nc.scalar.activation(
                                        out_sb[:, v * es:(v + 1) * es],
                                        psums[qb][:, v * es:(v + 1) * es],
                                        mybir.ActivationFunctionType.Identity,
                                        bias=bias_sb[:, 0:1],
                                        scale=1.0,
                                    )
                            if not skip_store:
                                store_eng.dma_start(
                                    out=out_d.ap()[:, qb * bpg:(qb + 1) * bpg, :],
                                    in_=out_sb,
                                )

    nc.compile()
    return nc


def _prep_host(x0, xk, W, b):
    """Host-side layout prep (no arithmetic): shard, transpose, replicate."""
    part = np.arange(128)
    hh = (part % HB)[None, :] + HB * np.arange(NHB)[:, None]   # [NHB, 128]
    mm = (part // HB)[None, :] + MG * np.arange(NMG)[:, None]  # [NMG, 128]

    Wr = W.reshape(C, H, M)
    w3 = np.empty((128, NCHUNK, C), ml_dtypes.bfloat16)
    for g in range(NMG):
        for j in range(NHB):
            w3[:, g * NHB + j, :] = Wr[:, hh[j], mm[g]].T.astype(
                ml_dtypes.bfloat16
            )
    w3 = np.ascontiguousarray(w3.reshape(128, NCHUNK * C))
    bias = np.ascontiguousarray(b.reshape(C, 1)).astype(np.float32)

    in_maps = []
    for k in range(N_CORES):
        x0s = x0[k * BC:(k + 1) * BC]            # [BC, M, D]
        xks = xk[k * BC:(k + 1) * BC]            # [BC, H, D]
        xk2 = (
            np.ascontiguousarray(xks.transpose(1, 0, 2))
            .reshape(H, COLS)
            .astype(ml_dtypes.bfloat16)
        )
        x02 = (
            np.ascontiguousarray(x0s.transpose(1, 0, 2))
            .reshape(M, COLS)
            .astype(ml_dtypes.bfloat16)
        )
        in_maps.append(
            {
                "xkrep_in": np.ascontiguousarray(xk2[hh]),
                "x0bc_in": np.ascontiguousarray(x02[mm]),
                "w3_in": w3,
                "bias_in": bias,
                "biasr_in": np.ascontiguousarray(
                    b.reshape(1, C)
                ).astype(ml_dtypes.bfloat16),
            }
        )
    return in_maps


def _run(in_maps, **kwargs):
    from concourse import bass_utils

    if "nc" not in _cache:
        _cache["nc"] = _build()
    return bass_utils.run_bass_kernel_spmd(
        _cache["nc"], in_maps, core_ids=list(range(N_CORES)), **kwargs
    )


def kernel(x0, xk, W, b, _bench=[None]):
    x0 = np.asarray(x0, dtype=np.float32)
    xk = np.asarray(xk, dtype=np.float32)
    W = np.asarray(W, dtype=np.float32)
    b = np.asarray(b, dtype=np.float32)
    in_maps = _prep_host(x0, xk, W, b)
    res = _run(in_maps)
    _bench[0] = res
    # per-core out is c-major [C, BC, D] (bf16); restore [BC, C, D], stack
    # cores, upcast to f32 on host
    out = np.concatenate(
        [np.transpose(np.asarray(r["out"], dtype=np.float32), (1, 0, 2))
         for r in res.results],
        axis=0,
    )
    return np.ascontiguousarray(out, dtype=np.float32)
